# revision 37
# baseline (speedup 1.0000x reference)
"""Multi-head attention (B=8, N=2048, C=320, H=5, D=64) on 8 Trainium2 cores.

Sharding: data-parallel over batch -- core b computes attention for x[b].
Weights replicated, no collectives.

Design notes:
  - fp16 operands everywhere, fp32 PSUM accumulation; x^T / w^T built by
    PE-mode transposes (identity matmul) streamed through the score PSUM
    rings during the DMA-bound prologue, with trivial warm-up matmuls
    keeping the PE clock gate (HAM) at full rate.
  - Score matmuls (K=64) run as row-tiled PAIRS: head A in array rows
    0-63, head B in rows 64-127, concurrently (512-wide streams so the
    1.2GHz LDWEIGHTS port does not bound). Head 4 self-pairs via
    duplicated qT/kT rows, processing two m-tiles per pass. Pair members
    must land in different PSUM banks (same-bank concurrent matmul
    writes lock the device).
  - Scores stream through alternating [128,3,512]/[128,2,512] fp32 PSUM
    units; one wide ACTIVATE per unit amortizes the ~0.5us fixed cost.
    Every 4th unit computes exp on the DVE instead, via a one-instruction
    Schraudolph approximation (int16(1477.32*s + 15315.3) bitcast to
    fp16, ~3% max err on ~18% of softmax weights, 4.6e-3 end-to-end),
    offloading the saturated ACT engine.
  - q pre-scaled by 1/sqrt(D) at eviction; exp runs with scale=1.
  - AV keeps a ones-column in V (M=65) so softmax denominators fall out
    of the accumulation for free; both heads pack into one 2-bank PSUM
    accumulator tile.
  - Each chunk front-drains the previous chunk's deferred AV/eviction
    jobs BEFORE its first scores: fills the chunk-boundary PE bubble and
    evicts the shared acc banks well ahead of the first start=True AV.
  - Attention chunk 0 starts after only THREE x groups: its first 8
    units (m-tiles 0-9) emit before the last x group, whose DMA, casts,
    transposes and k-chunk overlap that streaming (emit_attention is
    resumable via stop_at/cont). q-side qk chunks for n>=512 defer into
    the filler queue -- only k needs all m-tiles before chunk 0.
  - O^T normalization: denominator rows are DMA-spread across 128
    partitions for a cheap reciprocal, DMA'd back, broadcast by a K=1
    matmul and multiplied in. (A direct single-lane DVE reciprocal
    measures ~6.5us -- never do that.)
  - Projection PSUM-accumulates one K=128 matmul per head PAIR on top of
    the bias: steady-state tiles fuse the bias as a DVE tensor_tensor
    add at eviction (from a precomputed b_full broadcast tile); the four
    tail tiles instead seed PSUM with a K=1 bias matmul and evict on ACT
    (PE and ACT are idle at the drain, DVE is the tail bottleneck), and
    route through the then-idle score rings so they pipeline.
  - Tail order: pair 2's LAST chunk runs at the very end -- its selfpair
    norm is half-size, and pair 1's final-chunk norm DMA roundtrip
    overlaps pair 2's attention stream.
  - Engine split: x fp32->fp16 casts ride DVE (big planes) + GPSIMD
    (small plane) so cast latency, which gates the transposes and
    head-blocks the in-order PE, is the max not the sum; the LAST x
    group's transpose/qk evictions go to the still-idle ACT engine
    (they gate attention start); wTd dup copies ride GPSIMD. All other
    non-score work stays in the filler-job queue popped between a
    unit's scores and its (4-unit-lagged) AV matmuls.
  - PSUM budget: score units 3+2 banks + AV accumulator 2 + misc 1 = 8.
  - Measured (8-core SPMD, this trace setup): ~253us vs 258us baseline;
    PE busy ~209us (AV 84, scores+transposes 70, qk/v/scale 41, proj 9,
    warm 5), ACT ~140us, DVE ~121us. Known-dead ends: fp8 anywhere
    upstream of AV (softmax weight noise == quantization noise, ~4-8%
    rel err vs the 2e-2 gate); batching scores/AVs across units (trades
    ~125ns/transition PE overhead for in-order wait exposure, net loss);
    3-deep shared PSUM ring rotation (ring-free RATE, not depth, is
    what paces the PE; fillers in the rotation make it worse).
"""

import numpy as np

import concourse.bacc as bacc
import concourse.tile as tile
from concourse import mybir
from concourse.bass_utils import run_bass_kernel_spmd
from concourse.masks import make_identity

FP32 = mybir.dt.float32
FP16 = mybir.dt.float16
AF = mybir.ActivationFunctionType
ALU = mybir.AluOpType

B = 8
C = 320
H = 5
D = 64
SCALE = D ** -0.5
CT = [(0, 128), (128, 128), (256, 64)]   # c-contraction tiles
NW = 512                                  # attention n-chunk width


def build_program(N: int):
    nc = bacc.Bacc("TRN2", target_bir_lowering=False, debug=False)

    x_d = nc.dram_tensor("x", [N, C], FP32, kind="ExternalInput")
    wqkv_d = nc.dram_tensor("w_qkv", [3 * C, C], FP32, kind="ExternalInput")
    wproj_d = nc.dram_tensor("w_proj", [C, C], FP32, kind="ExternalInput")
    bproj_d = nc.dram_tensor("b_proj", [C], FP32, kind="ExternalInput")
    out_d = nc.dram_tensor("out", [N, C], FP32, kind="ExternalOutput")

    MT = N // 128          # seq tiles (16)
    NCH = N // NW          # attention chunks (4)
    XG = MT // 4           # x load groups

    with tile.TileContext(nc) as tc:
        with (
            tc.tile_pool(name="per", bufs=1) as per,
            tc.tile_pool(name="ld", bufs=4) as ld,
            tc.tile_pool(name="pt", bufs=13) as pt_pool,
            tc.tile_pool(name="yacc", bufs=4) as yacc_pool,
            tc.tile_pool(name="dst", bufs=6) as dst_pool,
            tc.tile_pool(name="hex", bufs=1, space="PSUM") as hex_pool,
            tc.tile_pool(name="acc", bufs=1, space="PSUM") as acc_pool,
            tc.tile_pool(name="ms", bufs=1, space="PSUM") as ms_pool,
        ):
            xT = per.tile([128, 3, N], FP16)      # plane ci = c block
            wT = per.tile([128, 3, 3 * C], FP16)  # w_qkv^T planes
            wTd = per.tile([128, 3, 256], FP16)   # head-4 dup stationaries q|k
            wpt = per.tile([128, 3, C], FP16)     # w_proj^T planes
            qT = per.tile([128, 3, N], FP16)      # pre-scaled by SCALE
            kT = per.tile([128, 3, N], FP16)
            v_sb = per.tile([128, MT, H * (D + 1)], FP16)
            oT = per.tile([128, 3, N], FP16)      # plane p: heads 2p | 2p+1
            b_row32 = per.tile([1, C], FP32)
            b_row = per.tile([1, C], FP16)
            b_full = per.tile([128, C], FP32)     # bias broadcast to all rows
            ones1 = per.tile([1, 128], FP16)
            nc.gpsimd.memset(ones1[:], 1.0)
            identity = per.tile([128, 128], FP32)
            make_identity(nc, identity[:])
            identity_h = per.tile([128, 128], FP16)
            nc.vector.tensor_copy(identity_h[:], identity[:])

            v_heads = v_sb[:].rearrange("p m (h e) -> p m h e", h=H)
            nc.gpsimd.memset(v_heads[:, :, :, D : D + 1], 1.0)
            # zero the unused half of oT plane 2 (head 4 has no partner) so
            # the pair-contraction projection matmul adds exact zeros
            nc.gpsimd.memset(oT[64:128, 2, :], 0.0)

            def ev_copy(dst, src, eng, scale=None):
                if eng == "act":
                    nc.scalar.activation(
                        dst, src, AF.Copy,
                        scale=1.0 if scale is None else scale,
                    )
                elif scale is None:
                    nc.vector.tensor_copy(dst, src)
                else:
                    nc.vector.tensor_scalar_mul(dst, src, scale)

            # -------- weights: load + cast + PE transpose ------------------
            # (dma_start_transpose is deadlock-guard-serialized against all
            # other DMA traffic by the framework -- way too slow here)
            def transpose3(dst3, src3, rp, idx, ev="vector"):
                # dst3[c, ci, r] = src3[r, ci, c] via 3 PE transposes through
                # one alternating hex/duo PSUM tile + a single wide eviction
                sp = hex_pool.tile([128, 3, 128], FP16,
                                   tag="h3" if idx % 2 == 0 else "h2")
                for ci, (c0, cp) in enumerate(CT):
                    nc.tensor.transpose(
                        sp[:cp, ci, :rp], src3[:rp, ci, :cp],
                        identity_h[:rp, :rp],
                    )
                ev_copy(dst3[:, :, :], sp[:, :, 0:rp], ev)

            def emit_w_tile(dst, src_d, wt, nrows, idx=0):
                r0 = wt * 128
                rp = min(128, nrows - r0)
                wnat = ld.tile([128, C], FP32, tag="wnat")
                nc.sync.dma_start(wnat[:rp, :], src_d.ap()[r0 : r0 + rp, :])
                wnat_h = ld.tile([128, 3, 128], FP16, tag="wnat_h")
                for ci, (c0, cp) in enumerate(CT):
                    nc.vector.tensor_copy(
                        wnat_h[:rp, ci, :cp], wnat[:rp, c0 : c0 + cp]
                    )
                transpose3(dst[:, :, r0 : r0 + rp], wnat_h, rp, idx)

            def emit_wtd():
                # head-4 stationaries duplicated into array rows 64-127
                for ci, (c0, cp) in enumerate(CT):
                    for half in range(2):
                        nc.gpsimd.tensor_copy(
                            wTd[:cp, ci, 64 * half : 64 * half + 64],
                            wT[:cp, ci, 256:320],
                        )
                        nc.gpsimd.tensor_copy(
                            wTd[:cp, ci, 128 + 64 * half : 192 + 64 * half],
                            wT[:cp, ci, C + 256 : C + 320],
                        )

            # -------- x: load + cast + PE transpose ------------------------
            x_re = x_d.ap().rearrange("(t p) c -> p t c", p=128)

            def emit_x_group(g):
                gn = min(4, MT - g * 4)
                xnat = ld.tile([128, 4, C], FP32, tag="xnat")
                nc.sync.dma_start(xnat[:, :gn, :], x_re[:, g * 4 : g * 4 + gn, :])
                xh = ld.tile([128, 4, 3, 128], FP16, tag="xh")
                for ci, (c0, cp) in enumerate(CT):
                    # split the fp32->fp16 cast: big planes on DVE, the small
                    # plane on GPSIMD, so cast latency (which gates the
                    # transposes) is the max of the two queues, not a sum
                    eng = nc.gpsimd if ci == 2 else nc.vector
                    eng.tensor_copy(
                        xh[:, :gn, ci, :cp], xnat[:, :gn, c0 : c0 + cp]
                    )
                for t in range(gn):
                    transpose3(
                        xT[:, :, (g * 4 + t) * 128 : (g * 4 + t + 1) * 128],
                        xh[:, t, :, :], 128, t,
                    )

            # -------- qkv projections --------------------------------------
            def emit_qk_chunk(dst, plane, s0, sw=512, ev="vector"):
                base = dst * C
                ps = ms_pool.tile([128, 512], FP32, tag="m")
                for ci, (c0, cp) in enumerate(CT):
                    if plane < 2:
                        lhsT = wT[
                            :cp, ci, base + plane * 128 : base + plane * 128 + 128
                        ]
                    else:
                        lhsT = wTd[:cp, ci, dst * 128 : dst * 128 + 128]
                    nc.tensor.matmul(
                        ps[:, :sw],
                        lhsT,
                        xT[:cp, ci, s0 : s0 + sw],
                        start=(ci == 0),
                        stop=(ci == 2),
                    )
                out = (qT if dst == 0 else kT)[:, plane, s0 : s0 + sw]
                ev_copy(out, ps[:, :sw], ev, scale=SCALE if dst == 0 else None)

            def emit_v_tile(mt):
                ps = ms_pool.tile([128, 512], FP32, tag="m")
                for ci, (c0, cp) in enumerate(CT):
                    nc.tensor.matmul(
                        ps[:, :C],
                        xT[:cp, ci, mt * 128 : (mt + 1) * 128],
                        wT[:cp, ci, 2 * C : 3 * C],
                        start=(ci == 0),
                        stop=(ci == 2),
                    )
                nc.vector.tensor_copy(
                    v_heads[:, mt, :, 0:D],
                    ps[:, :C].rearrange("p (h e) -> p h e", h=H),
                )

            # -------- attention --------------------------------------------
            def emit_attention(p, nci, jobs, stop_at=None, cont=None,
                               drain=True):
                """pair p: heads (2p, 2p+1); p==2: head 4 self-paired.

                stop_at/cont allow splitting one chunk across two calls so
                other emission (the last x group) can interleave: the first
                call returns a continuation dict instead of tail jobs."""
                n0 = nci * NW
                selfpair = p == 2
                if selfpair:
                    steps = [(2 * i, 2 * i + 1) for i in range(MT // 2)]
                else:
                    steps = [(i, i) for i in range(MT)]
                ntile = 2 * len(steps)

                def tile_info(g):
                    j, s = g // 2, g % 2
                    ma, mb = steps[j]
                    mt = ma if s == 0 else mb
                    a = 0 if selfpair else s
                    h = 2 * p + a
                    return mt, s, a, h

                if cont is None:
                    # drain the previous chunk's deferred AV/eviction jobs
                    # (and whatever else queued) BEFORE this chunk's first
                    # scores: fills the chunk-boundary bubble and gets the
                    # shared acc banks evicted well before this chunk's
                    # first start=True AV. (The very FIRST chunk skips this:
                    # its queue holds v tiles, which serialize ~1us each on
                    # the ms bank and would push attention start ~8us later;
                    # per-unit filler pops deliver them just-in-time.)
                    if drain:
                        for _ in range(8):
                            if jobs:
                                jobs.pop(0)()
                    acc = acc_pool.tile([65, 2, NW], FP32, tag="a")
                else:
                    acc = cont["acc"]

                def emit_av(ptt, tris):
                    for idx, g in tris:
                        mt, s, a, h = tile_info(g)
                        if selfpair:
                            first, last = g == 0, g == ntile - 1
                        else:
                            first, last = g == a, g == ntile - 2 + a
                        nc.tensor.matmul(
                            acc[0:65, a, :],
                            v_sb[:, mt, h * (D + 1) : (h + 1) * (D + 1)],
                            ptt[:, idx, :],
                            start=first,
                            stop=last,
                        )

                units = []
                g0, toggle = 0, True
                while g0 < ntile:
                    w = min(3 if toggle else 2, ntile - g0)
                    units.append(list(range(g0, g0 + w)))
                    g0 += w
                    toggle = not toggle

                # the FINAL chunk's denominator copy rides the then-idle
                # ACT engine so it runs in parallel with the DVE oT evict:
                # the spread DMA (tail critical path) fires ~0.6us earlier
                final = p == 2 and nci == NCH - 1

                def evict(state):
                    # O^T slices + denominators (staged at partition 0 so the
                    # later 1/denom broadcast matmul is legal)
                    dstg = dst_pool.tile([1, 2, NW], FP16, tag="dst")
                    if not selfpair:
                        ev_copy(dstg[:], acc[64:65, :, :], "vector")
                    else:
                        ev_copy(dstg[0:1, 0, :], acc[64:65, 0, :],
                                "act" if final else "vector")
                    nc.vector.tensor_copy(
                        oT[0:64, p, n0 : n0 + NW], acc[0:64, 0, :]
                    )
                    if not selfpair:
                        nc.vector.tensor_copy(
                            oT[64:128, p, n0 : n0 + NW], acc[0:64, 1, :]
                        )
                    state["dstg"] = dstg

                pends = [] if cont is None else cont["pends"]
                ui0 = 0 if cont is None else cont["ui"]
                for ui, tri in list(enumerate(units))[ui0:stop_at]:
                    w = len(tri)
                    sp = hex_pool.tile([128, w, NW], FP32,
                                       tag="h3" if w == 3 else "h2")
                    for idx, g in enumerate(tri):
                        mt, s, a, h = tile_info(g)
                        nc.tensor.matmul(
                            sp[:, idx, :],
                            kT[64 * s : 64 * s + 64, p, mt * 128 : mt * 128 + 128],
                            qT[64 * s : 64 * s + 64, p, n0 : n0 + NW],
                            start=True,
                            stop=True,
                        )
                    ptt = pt_pool.tile([128, 3, NW], FP16, tag="pt")
                    if ui % 4 == 3:
                        # Schraudolph fp16 exp on the DVE: offloads ~1/4 of
                        # the softmax from the saturated ACT engine (~3% max
                        # err on these units' weights only)
                        with nc.allow_low_precision(
                            reason="approx exp offload for softmax"
                        ):
                            nc.vector.tensor_scalar(
                                ptt[:, :w, :]
                                .rearrange("p a b -> p (a b)")
                                .bitcast(mybir.dt.int16),
                                sp[:].rearrange("p a b -> p (a b)"),
                                1477.3196,
                                15315.3,
                                ALU.mult,
                                ALU.add,
                            )
                    else:
                        nc.scalar.activation(
                            ptt[:, :w, :].rearrange("p a b -> p (a b)"),
                            sp[:].rearrange("p a b -> p (a b)"),
                            AF.Exp,
                        )
                    # fillers go between the scores and the (lagged) AV so
                    # their stalls never delay the next exp; drain double
                    # while the startup backlog (v tiles etc.) is deep
                    for _ in range(2 if len(jobs) > 16 else 1):
                        if jobs:
                            jobs.pop(0)()
                    pends.append((ptt, list(enumerate(tri))))
                    if len(pends) > 4:
                        emit_av(*pends.pop(0))

                if stop_at is not None:
                    return {"acc": acc, "pends": pends, "ui": stop_at}

                # remaining AVs + eviction defer into the next chunk's fillers
                state = {}
                out = [
                    (lambda pd=pd: emit_av(*pd)) for pd in pends
                ]
                out.append(lambda: evict(state))
                return state, out

            # -------- O^T normalization (in-place) -------------------------
            # The denominator rows live on one partition; a direct DVE
            # reciprocal there runs on a single lane (~6.5us measured).
            # Instead DMA the row across 128 partitions, reciprocal there
            # (fp16 out), DMA back, then broadcast via a K=1 matmul and
            # scale O^T in place.
            def emit_spread(att_state, state):
                dstg = att_state["dstg"]
                dT = yacc_pool.tile([128, 8], FP16, tag="dT")
                nc.sync.dma_start(
                    dT[:, :], dstg[0:1, :, :].rearrange("x a b -> x (a b)")
                )
                state["dT"] = dT

            def emit_recip(state):
                rT = yacc_pool.tile([128, 8], FP16, tag="rT")
                with nc.allow_low_precision(
                    reason="softmax 1/denom as fp16 broadcast operand"
                ):
                    nc.vector.reciprocal(rT[:], state["dT"][:])
                state["rT"] = rT

            def emit_unspread(state):
                drow = yacc_pool.tile([1, 2, NW], FP16, tag="drow")
                nc.sync.dma_start(
                    drow[0:1, :, :].rearrange("x a b -> x (a b)"), state["rT"][:]
                )
                state["drow"] = drow

            def emit_scale(state, p, half, nci):
                n0 = nci * NW
                ps = ms_pool.tile([128, 512], FP32, tag="m")
                nc.tensor.matmul(
                    ps[0:64, :NW],
                    ones1[0:1, 0:64],
                    state["drow"][0:1, half, :],
                    start=True,
                    stop=True,
                )
                sl = oT[64 * half : 64 * half + 64, p, n0 : n0 + NW]
                nc.vector.tensor_tensor(sl, sl, ps[0:64, :NW], ALU.mult)

            # -------- projection (PSUM-accumulated) ------------------------
            # One K=128 matmul per head PAIR (oT stacks the pair's d dims on
            # the partition axis); bias added at eviction from b_full. All
            # chain members share row groups, so they serialize in the PE --
            # never two concurrent writers on one PSUM bank. Plane-2 rows
            # 64-127 of oT and wpt are zeroed.
            def emit_proj_tile(gt, ring=None):
                # steady state: ms bank + fused DVE bias-add at eviction.
                # tail (ring-routed): bias-seed matmul + ACT eviction -- PE
                # and ACT are idle at the drain, DVE is the tail bottleneck
                if ring is None:
                    ypt = ms_pool.tile([128, 512], FP32, tag="m")
                    yp = ypt[:, :C]
                else:
                    ypt = hex_pool.tile([128, 1, 512], FP32, tag=ring)
                    yp = ypt[:, 0, :C]
                    nc.tensor.matmul(
                        yp, ones1[:], b_row[:], start=True, stop=False
                    )
                for p in range(3):
                    nc.tensor.matmul(
                        yp,
                        oT[:, p, gt * 128 : (gt + 1) * 128],
                        wpt[:, p, :],
                        start=(p == 0 and ring is None),
                        stop=(p == 2),
                    )
                yout = yacc_pool.tile([128, C], FP32, tag="acc")
                if ring is None:
                    nc.vector.tensor_tensor(yout[:], yp, b_full[:], ALU.add)
                else:
                    ev_copy(yout[:], yp, "act")
                nc.sync.dma_start(out_d.ap()[gt * 128 : (gt + 1) * 128, :], yout[:])

            def emit_proj_begin(gt, ypt, slot):
                # bias seed + pair-0/1 planes: these do NOT depend on the
                # final selfpair norm, so they fill the PE bubble while its
                # spread/recip/unspread DMA roundtrip is in flight
                yp = ypt[:, slot, :C]
                # CLOSED group (stop=True): holding a group open across the
                # other begins/scale serializes the PE globally (+48us!).
                # The end-half accumulates on top via start=False.
                nc.tensor.matmul(yp, ones1[:], b_row[:], start=True, stop=False)
                for p in range(2):
                    nc.tensor.matmul(
                        yp,
                        oT[:, p, gt * 128 : (gt + 1) * 128],
                        wpt[:, p, :],
                        start=False,
                        stop=(p == 1),
                    )

            def emit_proj_end(gt, ypt, slot):
                yp = ypt[:, slot, :C]
                nc.tensor.matmul(
                    yp,
                    oT[:, 2, gt * 128 : (gt + 1) * 128],
                    wpt[:, 2, :],
                    start=False,
                    stop=True,
                )
                yout = yacc_pool.tile([128, C], FP32, tag="acc")
                ev_copy(yout[:], yp, "act")
                nc.sync.dma_start(out_d.ap()[gt * 128 : (gt + 1) * 128, :], yout[:])

            # -------- emission schedule ------------------------------------
            warm = acc_pool.tile([65, 2, NW], FP32, tag="a")

            def warm_mm(n):
                # trivial matmuls keep the PE clock (HAM) warm: PE-mode
                # transposes don't count as activity for the clock gate
                for _ in range(n):
                    nc.tensor.matmul(
                        warm[0:65, 0, 0:128], ones1[0:1, 0:65], ones1[0:1, :],
                        start=True, stop=True,
                    )

            warm_mm(12)
            nc.sync.dma_start(
                b_row32[:], bproj_d.ap().rearrange("(a c) -> a c", a=1)
            )
            nc.vector.tensor_copy(b_row[:], b_row32[:])
            for wt in range(8):
                emit_w_tile(wT, wqkv_d, wt, 3 * C, idx=wt)
                warm_mm(2)
            # bias broadcast: one K=1 matmul, evicted fp32 for fused adds
            bf = ms_pool.tile([128, 512], FP32, tag="m")
            nc.tensor.matmul(
                bf[:, :C], ones1[:], b_row[:], start=True, stop=True
            )
            nc.vector.tensor_copy(b_full[:], bf[:, :C])
            for g in range(XG - 1):
                emit_x_group(g)
                warm_mm(2)
                # k side: attention chunk 0 needs ALL m-tiles; q side: only
                # columns 0-511 (later q chunks defer into the filler queue)
                emit_qk_chunk(1, 0, g * 512)
                if g == 0:
                    emit_qk_chunk(0, 0, 0)

            # start attention chunk 0 with the 8 units that need only
            # x-groups 0-2 (m-tiles 0-9); the last x group's DMA, casts,
            # transposes and k-chunk then overlap this streaming instead of
            # serializing in front of it
            jobs = [lambda m=mt: emit_v_tile(m) for mt in range(10)]
            att_cont = emit_attention(0, 0, jobs, stop_at=8)
            emit_x_group(XG - 1)
            emit_qk_chunk(1, 0, (XG - 1) * 512)
            emit_wtd()

            jobs.extend(lambda m=mt: emit_v_tile(m) for mt in range(10, MT))
            for s0 in range(512, N, 512):
                jobs.append(lambda s=s0: emit_qk_chunk(0, 0, s))
            for wt in range(3):
                jobs.append(lambda w=wt: emit_w_tile(wpt, wproj_d, w, C, idx=w))
            # overwrite the pad junk the wproj transposes leave in plane 2
            jobs.append(lambda: nc.gpsimd.memset(wpt[64:128, 2, :], 0.0))
            for s0 in range(0, N, 512):
                jobs.append(lambda s=s0: emit_qk_chunk(0, 2, s))
                jobs.append(lambda s=s0: emit_qk_chunk(1, 2, s))
            for s0 in range(0, N, 512):
                jobs.append(lambda s=s0: emit_qk_chunk(0, 1, s))
                jobs.append(lambda s=s0: emit_qk_chunk(1, 1, s))

            def norm_jobs(att_state, p, nci):
                state = {}
                out = [
                    lambda: emit_spread(att_state, state),
                    lambda: emit_recip(state),
                    lambda: emit_unspread(state),
                    lambda: emit_scale(state, p, 0, nci),
                ]
                if p != 2:
                    out.append(lambda: emit_scale(state, p, 1, nci))
                return out

            # pair order 0, 2, 1: the projection (which needs all pairs of a
            # chunk) then interleaves into the LAST pair's stream, whose
            # chunks have light attention load
            for nci in range(NCH):
                st, tail = emit_attention(
                    0, nci, jobs, cont=att_cont if nci == 0 else None
                )
                jobs[0:0] = tail
                jobs.extend(norm_jobs(st, 0, nci))
            for nci in range(NCH - 1):
                st, tail = emit_attention(2, nci, jobs)
                jobs[0:0] = tail
                jobs.extend(norm_jobs(st, 2, nci))
            for nci in range(NCH):
                st, tail = emit_attention(1, nci, jobs)
                jobs[0:0] = tail
                jobs.extend(norm_jobs(st, 1, nci))
                if nci < NCH - 1:
                    jobs.extend(
                        lambda g=nci * 4 + t: emit_proj_tile(g)
                        for t in range(4)
                    )
            # pair 2's last chunk runs LAST: its selfpair norm is half-size,
            # and pair 1's final-chunk norm roundtrip overlaps this stream
            st, tail = emit_attention(2, NCH - 1, jobs)
            jobs[0:0] = tail
            # final projections split: bias+planes-0/1 accumulate into the
            # then-idle score rings DURING the selfpair norm's DMA
            # roundtrip; plane 2 + eviction follow the scale. Tiles 12-14
            # share the 3-slot h3 ring, tile 15 the h2 ring -- allocated
            # once so the slots don't WAR-serialize against each other.
            pj3 = hex_pool.tile([128, 3, NW], FP32, tag="h3", name="pj3")
            pj2 = hex_pool.tile([128, 2, NW], FP32, tag="h2", name="pj2")
            plan = [(12, pj3, 0), (13, pj3, 1), (14, pj3, 2), (15, pj2, 0)]
            nstate = {}
            jobs.append(lambda: emit_spread(st, nstate))
            jobs.append(lambda: emit_recip(nstate))
            jobs.append(lambda: emit_unspread(nstate))
            for gt, tt, s in plan:
                jobs.append(lambda g=gt, t=tt, sl=s: emit_proj_begin(g, t, sl))
            jobs.append(lambda: emit_scale(nstate, 2, 0, NCH - 1))
            for gt, tt, s in plan:
                jobs.append(lambda g=gt, t=tt, sl=s: emit_proj_end(g, t, sl))
            while jobs:
                jobs.pop(0)()

    nc.compile()
    return nc


_cache = {}


def _get_program(N: int):
    if N not in _cache:
        _cache[N] = build_program(N)
    return _cache[N]


def kernel(x, w_qkv, w_proj, b_proj):
    x = np.ascontiguousarray(np.asarray(x, dtype=np.float32))
    w_qkv = np.ascontiguousarray(np.asarray(w_qkv, dtype=np.float32))
    w_proj = np.ascontiguousarray(np.asarray(w_proj, dtype=np.float32))
    b_proj = np.ascontiguousarray(np.asarray(b_proj, dtype=np.float32))
    Bx, N, Cx = x.shape
    assert Bx == B and Cx == C, (x.shape,)

    nc = _get_program(N)
    in_maps = [
        {"x": x[b], "w_qkv": w_qkv, "w_proj": w_proj, "b_proj": b_proj}
        for b in range(B)
    ]
    res = run_bass_kernel_spmd(nc, in_maps, core_ids=list(range(B)))
    return np.stack([res.results[b]["out"] for b in range(B)], axis=0)
